# revision 7
# baseline (speedup 1.0000x reference)
"""Trainium2 Bass kernel for a 2-layer LSTM + MLP head (LyricSTM).

Strategy: data-parallel over batch across 8 NeuronCores (weights replicated,
no collectives). Per core everything is kept transposed as
[feature/gate partitions, batch free] so the time recurrence is layout-stable:

  gates1.T [3072, 64] = W1c.T.T @ [x_t.T ; h1.T]      (weights-stationary MMs)
  gates2.T [3072, 64] = W2c.T.T @ [h1.T ; h2.T]
  head: out2.T [128, 64] = l2.T.T @ selu(l1.T.T @ [h2.T ; gen.T])

Numerics tricks (all host-folded, zero extra device ops):
  - sigmoid(x) = 0.5*tanh(x/2)+0.5 -> keep states as hh=2h, D=2c and scale the
    weight columns that consume hh by 0.5 on the host. Only `tanh` + `exp` are
    used on ACT, so a single activation-table set is loaded once.
  - selu(u) = s*(relu(u) + min(alpha*(e^u - 1), 0)); alpha folded into the Exp
    bias (exp(u + ln(alpha))), s folded into l2 weights.

Matmuls run in bf16 (fp32 PSUM accumulation); element-wise math in fp32.
"""

import math

import numpy as np
import ml_dtypes

import concourse.bass as bass
import concourse.tile as tile
from concourse import bacc, mybir
from concourse.bass import ds
from concourse.bass import _add_dep_helper as add_dep_helper

F, G = 128, 12          # char features / genre features (x has F+G=140)
H = 768                 # lstm hidden
NG = 4 * H              # 3072 gates
D1 = 512                # mlp hidden
A = 128                 # output alphabet
T_FULL, B_FULL = 512, 512
N_CORES = 8

BF16 = mybir.dt.bfloat16
F32 = mybir.dt.float32
AF = mybir.ActivationFunctionType
ALU = mybir.AluOpType

SELU_ALPHA = 1.6732632423543772
SELU_SCALE = 1.0507009873554805
LN_ALPHA = math.log(SELU_ALPHA)

# gate blocks of 128 rows, ordered [i, f, o, g] so the three sigmoid gates are
# contiguous in the psum free dim (single batched tanh(x/2) op covers them).
_PERM = np.concatenate([
    np.arange(0, H),            # i
    np.arange(H, 2 * H),        # f
    np.arange(3 * H, 4 * H),    # o
    np.arange(2 * H, 3 * H),    # g
])

MI, MF, MO, MG = 0, 6, 12, 18   # m-block index (of 24) where each gate starts


def build_nc(T: int, BL: int, unroll: int = 4):
    """Build + compile the per-core Bass program. BL = per-core batch."""
    nc = bacc.Bacc("TRN2", target_bir_lowering=False, debug=False,
                   enable_asserts=False)

    KC1 = 7    # k chunks for layer1: x(1) + h1(6)
    KC2 = 12   # layer2: h1(6) + h2(6)
    M = NG // 128          # 24 gate m-blocks
    HB = H // 128          # 6 h chunks
    GIFO = 3 * H           # 2304 = i,f,o region in the permuted gate dim

    # ---- DRAM I/O ----
    xT = nc.dram_tensor("xT", [T * (F + G), BL], BF16, kind="ExternalInput").ap()
    w1 = nc.dram_tensor("w1", [KC1 * 128, NG], BF16, kind="ExternalInput").ap()
    w2 = nc.dram_tensor("w2", [KC2 * 128, NG], BF16, kind="ExternalInput").ap()
    l1h = nc.dram_tensor("l1h", [H, D1], BF16, kind="ExternalInput").ap()
    l1g = nc.dram_tensor("l1g", [G, D1], BF16, kind="ExternalInput").ap()
    l2 = nc.dram_tensor("l2", [D1, A], BF16, kind="ExternalInput").ap()
    b1bc = nc.dram_tensor("b1bc", [128, M * BL], F32, kind="ExternalInput").ap()
    b2bc = nc.dram_tensor("b2bc", [128, M * BL], F32, kind="ExternalInput").ap()
    l1bc = nc.dram_tensor("l1bc", [128, 4 * BL], F32, kind="ExternalInput").ap()
    l2b = nc.dram_tensor("l2b", [128, 1], F32, kind="ExternalInput").ap()
    out = nc.dram_tensor("out", [T * A, BL], F32, kind="ExternalOutput").ap()

    with tile.TileContext(nc) as tc:
        with (
            tc.tile_pool(name="wpool", bufs=1) as wpool,
            tc.tile_pool(name="state", bufs=1) as state,
            tc.tile_pool(name="xin", bufs=2 * unroll) as xin,
            tc.tile_pool(name="work", bufs=2) as work,
            tc.tile_pool(name="ops", bufs=2) as ops,
            tc.tile_pool(name="outr", bufs=4) as outr,
            tc.tile_pool(name="pg", bufs=2, space="PSUM") as pg,
            tc.tile_pool(name="ph", bufs=1, space="PSUM") as ph,
        ):
            # ---- resident weights ----
            w1_sb = wpool.tile([128, KC1 * NG], BF16, tag="w1")
            for k in range(KC1):
                nc.sync.dma_start(w1_sb[:, k * NG:(k + 1) * NG],
                                  w1[k * 128:(k + 1) * 128, :])
            w2_sb = wpool.tile([128, KC2 * NG], BF16, tag="w2")
            for k in range(KC2):
                nc.sync.dma_start(w2_sb[:, k * NG:(k + 1) * NG],
                                  w2[k * 128:(k + 1) * 128, :])
            l1h_sb = wpool.tile([128, HB * D1], BF16, tag="l1h")
            for k in range(HB):
                nc.sync.dma_start(l1h_sb[:, k * D1:(k + 1) * D1],
                                  l1h[k * 128:(k + 1) * 128, :])
            l1g_sb = wpool.tile([G, D1], BF16, tag="l1g")
            nc.sync.dma_start(l1g_sb[:], l1g[:])
            l2_sb = wpool.tile([128, 4 * A], BF16, tag="l2")
            for k in range(4):
                nc.sync.dma_start(l2_sb[:, k * A:(k + 1) * A],
                                  l2[k * 128:(k + 1) * 128, :])
            b1_sb = wpool.tile([128, M * BL], F32, tag="b1")
            nc.sync.dma_start(b1_sb[:], b1bc[:])
            b2_sb = wpool.tile([128, M * BL], F32, tag="b2")
            nc.sync.dma_start(b2_sb[:], b2bc[:])
            l1b_sb = wpool.tile([128, 4 * BL], F32, tag="l1b")
            nc.sync.dma_start(l1b_sb[:], l1bc[:])
            l2b_sb = wpool.tile([128, 1], F32, tag="l2b")
            nc.sync.dma_start(l2b_sb[:], l2b[:])
            lnal_sb = wpool.tile([128, 1], F32, tag="lna")
            nc.vector.memset(lnal_sb[:], LN_ALPHA)

            # ---- recurrent state (hh = 2h bf16, D = 2c fp32) ----
            h1_sb = state.tile([128, HB * BL], BF16, tag="h1")
            h2_sb = state.tile([128, HB * BL], BF16, tag="h2")
            d1_sb = state.tile([128, HB * BL], F32, tag="d1")
            d2_sb = state.tile([128, HB * BL], F32, tag="d2")
            for t_ in (h1_sb, h2_sb, d1_sb, d2_sb):
                nc.vector.memset(t_[:], 0.0)

            def lstm_layer(gp, b_sb, d_sb, h_out):
                """gates psum [128, M*BL] -> update D state, write hh (2h)."""
                u = work.tile([128, M * BL], F32, tag="u")
                # u = gates + bias   (i,f,o then g regions, one op)
                nc.vector.scalar_tensor_tensor(
                    u[:], gp[:], 0.0, b_sb[:], ALU.bypass, ALU.add)
                tt = work.tile([128, M * BL], F32, tag="tt")
                ifo = (GIFO // 128) * BL
                # tanh(x/2) for i,f,o ; tanh(x) for g
                nc.scalar.activation(tt[:, 0:ifo], u[:, 0:ifo], AF.Tanh, scale=0.5)
                nc.scalar.activation(tt[:, ifo:], u[:, ifo:], AF.Tanh)
                hbl = HB * BL
                t_i = tt[:, 0:hbl]
                t_f = tt[:, hbl:2 * hbl]
                t_o = tt[:, 2 * hbl:3 * hbl]
                t_g = tt[:, 3 * hbl:4 * hbl]
                a_t = work.tile([128, hbl], F32, tag="a")
                b_t = work.tile([128, hbl], F32, tag="b")
                # A = (1+t_f) * D ; B = (1+t_i) * tanh(g) ; D = 0.5A + B
                nc.vector.scalar_tensor_tensor(
                    a_t[:], t_f, 1.0, d_sb[:], ALU.add, ALU.mult)
                nc.vector.scalar_tensor_tensor(
                    b_t[:], t_i, 1.0, t_g, ALU.add, ALU.mult)
                nc.vector.scalar_tensor_tensor(
                    d_sb[:], a_t[:], 0.5, b_t[:], ALU.mult, ALU.add)
                tc_t = work.tile([128, hbl], F32, tag="tc")
                nc.scalar.activation(tc_t[:], d_sb[:], AF.Tanh, scale=0.5)
                # hh = (1+t_o) * tanh(c)  == 2h
                nc.vector.scalar_tensor_tensor(
                    h_out[:], t_o, 1.0, tc_t[:], ALU.add, ALU.mult)

            def chain_groups(mms_last, mms_first, per_bank):
                """order psum accumulation groups sharing a bank."""
                for i in range(1, len(mms_first)):
                    if (i % per_bank) != 0:
                        add_dep_helper(mms_first[i].ins, mms_last[i - 1].ins,
                                       sync=False,
                                       reason="psum bank group order")

            def step(t):
                xf = xin.tile([128, BL], BF16, tag="xf")
                nc.sync.dma_start(xf[:], xT[ds(t * (F + G), 128), :])
                xg = xin.tile([G, BL], BF16, tag="xg")
                nc.sync.dma_start(xg[:], xT[ds(t * (F + G) + 128, G), :])

                # ---- layer 1 gates ----
                g1 = pg.tile([128, M * BL], F32, tag="gates")
                firsts, lasts = [], []
                for m in range(M):
                    o_ap = g1[:, m * BL:(m + 1) * BL]
                    for k in range(KC1):
                        rhs = xf[:] if k == 0 else h1_sb[:, (k - 1) * BL:k * BL]
                        mm = nc.tensor.matmul(
                            o_ap, w1_sb[:, k * NG + m * 128: k * NG + (m + 1) * 128],
                            rhs, start=(k == 0), stop=(k == KC1 - 1))
                        if k == 0:
                            firsts.append(mm)
                        if k == KC1 - 1:
                            lasts.append(mm)
                chain_groups(lasts, firsts, per_bank=(2048 // 4) // BL)
                lstm_layer(g1, b1_sb, d1_sb, h1_sb)

                # ---- layer 2 gates ----
                g2 = pg.tile([128, M * BL], F32, tag="gates")
                firsts, lasts = [], []
                for m in range(M):
                    o_ap = g2[:, m * BL:(m + 1) * BL]
                    for k in range(KC2):
                        h_sb = h1_sb if k < HB else h2_sb
                        kk = k if k < HB else k - HB
                        rhs = h_sb[:, kk * BL:(kk + 1) * BL]
                        mm = nc.tensor.matmul(
                            o_ap, w2_sb[:, k * NG + m * 128: k * NG + (m + 1) * 128],
                            rhs, start=(k == 0), stop=(k == KC2 - 1))
                        if k == 0:
                            firsts.append(mm)
                        if k == KC2 - 1:
                            lasts.append(mm)
                chain_groups(lasts, firsts, per_bank=(2048 // 4) // BL)
                lstm_layer(g2, b2_sb, d2_sb, h2_sb)

                # ---- MLP head ----
                p1 = ph.tile([128, 4 * BL], F32, tag="p1")
                firsts, lasts = [], []
                for m in range(4):
                    o_ap = p1[:, m * BL:(m + 1) * BL]
                    for k in range(HB + 1):
                        if k < HB:
                            mm = nc.tensor.matmul(
                                o_ap,
                                l1h_sb[:, k * D1 + m * 128: k * D1 + (m + 1) * 128],
                                h2_sb[:, k * BL:(k + 1) * BL],
                                start=(k == 0), stop=False)
                        else:
                            mm = nc.tensor.matmul(
                                o_ap, l1g_sb[:, m * 128:(m + 1) * 128], xg[:],
                                start=False, stop=True)
                        if k == 0:
                            firsts.append(mm)
                        if k == HB:
                            lasts.append(mm)
                chain_groups(lasts, firsts, per_bank=(2048 // 4) // BL)
                u = ops.tile([128, 4 * BL], F32, tag="hu")
                nc.vector.scalar_tensor_tensor(
                    u[:], p1[:], 0.0, l1b_sb[:], ALU.bypass, ALU.add)
                e = ops.tile([128, 4 * BL], F32, tag="he")
                nc.scalar.activation(e[:], u[:], AF.Exp, bias=lnal_sb[:])
                tm = ops.tile([128, 4 * BL], F32, tag="ht")
                nc.vector.tensor_scalar(tm[:], e[:], -SELU_ALPHA, 0.0,
                                        ALU.add, ALU.min)
                r = ops.tile([128, 4 * BL], F32, tag="hr")
                nc.vector.tensor_scalar(r[:], u[:], 0.0, None, ALU.max)
                s1 = ops.tile([128, 4 * BL], BF16, tag="hs")
                nc.vector.tensor_tensor(s1[:], r[:], tm[:], ALU.add)

                p2 = ph.tile([128, BL], F32, tag="p2")
                for k in range(4):
                    nc.tensor.matmul(
                        p2[:], l2_sb[:, k * A:(k + 1) * A],
                        s1[:, k * BL:(k + 1) * BL],
                        start=(k == 0), stop=(k == 3))
                o_sb = outr.tile([128, BL], F32, tag="ho")
                nc.scalar.activation(o_sb[:], p2[:], AF.Identity,
                                     bias=l2b_sb[:])
                nc.sync.dma_start(out[ds(t * A, A), :], o_sb[:])

            assert T % unroll == 0
            if T // unroll > 1:
                with tc.For_i(0, T, unroll,
                              hint_engines=(mybir.EngineType.PE,)) as iv:
                    for u_ in range(unroll):
                        step(iv + u_)
            else:
                for t in range(T):
                    step(t)

    nc.compile()
    return nc


def _host_pack(inputs, T, BL, cores):
    """Build the per-core in_maps (shared weight arrays + per-core x slice)."""
    f32 = np.float32
    bf16 = ml_dtypes.bfloat16
    W_ih1 = np.asarray(inputs["W_ih1"], f32)
    W_hh1 = np.asarray(inputs["W_hh1"], f32)
    W_ih2 = np.asarray(inputs["W_ih2"], f32)
    W_hh2 = np.asarray(inputs["W_hh2"], f32)
    l1_w = np.asarray(inputs["l1_w"], f32)
    l2_w = np.asarray(inputs["l2_w"], f32)
    b1 = (np.asarray(inputs["b_ih1"], f32) + np.asarray(inputs["b_hh1"], f32))[_PERM]
    b2 = (np.asarray(inputs["b_ih2"], f32) + np.asarray(inputs["b_hh2"], f32))[_PERM]
    l1_b = np.asarray(inputs["l1_b"], f32)
    l2_b = np.asarray(inputs["l2_b"], f32)

    w1 = np.ascontiguousarray(
        np.concatenate([W_ih1, 0.5 * W_hh1], axis=1)[_PERM].T).astype(bf16)
    w2 = np.ascontiguousarray(
        np.concatenate([0.5 * W_ih2, 0.5 * W_hh2], axis=1)[_PERM].T).astype(bf16)
    l1h = np.ascontiguousarray((0.5 * l1_w[:, :H]).T).astype(bf16)
    l1g = np.ascontiguousarray(l1_w[:, H:].T).astype(bf16)
    l2 = np.ascontiguousarray((SELU_SCALE * l2_w).T).astype(bf16)

    def bcast(b, nblk):
        return np.ascontiguousarray(
            np.broadcast_to(b.reshape(nblk, 128).T[:, :, None],
                            (128, nblk, BL)).reshape(128, nblk * BL)).astype(f32)

    b1bc = bcast(b1, NG // 128)
    b2bc = bcast(b2, NG // 128)
    l1bc = bcast(l1_b, 4)
    l2bc = np.ascontiguousarray(l2_b[:, None]).astype(f32)

    x = np.asarray(inputs["x"], f32)[:T]
    in_maps = []
    for c in cores:
        xc = x[:, c * BL:(c + 1) * BL, :]                      # [T, BL, 140]
        xT = np.ascontiguousarray(xc.transpose(0, 2, 1)).reshape(
            T * (F + G), BL).astype(bf16)
        in_maps.append({
            "xT": xT, "w1": w1, "w2": w2, "l1h": l1h, "l1g": l1g, "l2": l2,
            "b1bc": b1bc, "b2bc": b2bc, "l1bc": l1bc, "l2b": l2bc,
        })
    return in_maps


_CACHE = {}


def _get_nc(T, BL, unroll=4):
    key = (T, BL, unroll)
    if key not in _CACHE:
        _CACHE[key] = build_nc(T, BL, unroll)
    return _CACHE[key]


class _PjrtRunner:
    """run_bass_via_pjrt with a cached jitted executable + timing support."""

    def __init__(self, nc, n_cores):
        import jax
        from jax.experimental.shard_map import shard_map
        from jax.sharding import Mesh, PartitionSpec
        from concourse import mybir as mb
        from concourse.bass2jax import (_bass_exec_p, install_neuronx_cc_hook,
                                        partition_id_tensor)

        install_neuronx_cc_hook()
        self.n_cores = n_cores
        part_name = (nc.partition_id_tensor.name
                     if nc.partition_id_tensor else None)
        in_names, out_names, out_avals, zero_outs = [], [], [], []
        for alloc in nc.m.functions[0].allocations:
            if not isinstance(alloc, mb.MemoryLocationSet):
                continue
            name = alloc.memorylocations[0].name
            if alloc.kind == "ExternalInput":
                if name != part_name:
                    in_names.append(name)
            elif alloc.kind == "ExternalOutput":
                out_names.append(name)
                shape = tuple(alloc.tensor_shape)
                dtype = mb.dt.np(alloc.dtype)
                out_avals.append(jax.core.ShapedArray(shape, dtype))
                zero_outs.append(np.zeros(shape, dtype))
        self.in_names, self.out_names = in_names, out_names
        self.out_avals, self.zero_outs = out_avals, zero_outs
        n_params, n_outs = len(in_names), len(out_avals)
        all_names = list(in_names + out_names)
        if part_name is not None:
            all_names.append(part_name)
        all_names = tuple(all_names)

        def _body(*args):
            operands = list(args)
            if part_name is not None:
                operands.append(partition_id_tensor())
            outs = _bass_exec_p.bind(
                *operands, out_avals=tuple(out_avals), in_names=all_names,
                out_names=tuple(out_names), lowering_input_output_aliases=(),
                sim_require_finite=True, sim_require_nnan=True, nc=nc)
            return tuple(outs)

        devices = jax.devices()[:n_cores]
        assert len(devices) == n_cores
        self.mesh = Mesh(np.asarray(devices), ("core",))
        in_specs = (PartitionSpec("core"),) * (n_params + n_outs)
        out_specs = (PartitionSpec("core"),) * n_outs
        # no donation: lets the same device buffers be reused across timing
        # repeats (outputs are fully overwritten by the kernel each run)
        self.fn = jax.jit(shard_map(_body, mesh=self.mesh, in_specs=in_specs,
                                    out_specs=out_specs, check_rep=False))
        self.jax = jax

    def place(self, in_maps):
        """device_put the concatenated per-core inputs once."""
        import jax
        from jax.sharding import NamedSharding, PartitionSpec
        sh = NamedSharding(self.mesh, PartitionSpec("core"))
        args = []
        for name in self.in_names:
            cat = np.concatenate([np.asarray(m[name]) for m in in_maps], axis=0)
            args.append(jax.device_put(cat, sh))
        for z in self.zero_outs:
            cat = np.zeros((self.n_cores * z.shape[0], *z.shape[1:]), z.dtype)
            args.append(jax.device_put(cat, sh))
        return args

    def run(self, args):
        outs = self.fn(*args)
        self.jax.block_until_ready(outs)
        return outs

    def results(self, outs):
        res = []
        for c in range(self.n_cores):
            res.append({
                name: np.asarray(outs[i]).reshape(
                    self.n_cores, *self.out_avals[i].shape)[c]
                for i, name in enumerate(self.out_names)})
        return res


_RUNNER_CACHE = {}


def _get_runner(T, BL, unroll, n_cores):
    key = (T, BL, unroll, n_cores)
    if key not in _RUNNER_CACHE:
        _RUNNER_CACHE[key] = _PjrtRunner(_get_nc(T, BL, unroll), n_cores)
    return _RUNNER_CACHE[key]


def run_cores(inputs, T=T_FULL, BL=B_FULL // N_CORES, cores=None, unroll=4,
              time_repeats=0):
    import time as _time
    if cores is None:
        cores = list(range(N_CORES))
    runner = _get_runner(T, BL, unroll, len(cores))
    in_maps = _host_pack(inputs, T, BL, cores)
    args = runner.place(in_maps)
    outs = runner.run(args)
    times = []
    for _ in range(time_repeats):
        t0 = _time.perf_counter()
        outs = runner.run(args)
        times.append(_time.perf_counter() - t0)
    results = runner.results(outs)
    o = []
    for c in range(len(cores)):
        o.append(np.ascontiguousarray(
            results[c]["out"].reshape(T, A, BL).transpose(0, 2, 1)))
    return o, times


def kernel(**inputs) -> np.ndarray:
    BL = B_FULL // N_CORES
    outs, _ = run_cores(inputs)
    full = np.empty((T_FULL, B_FULL, A), np.float32)
    for c in range(N_CORES):
        full[:, c * BL:(c + 1) * BL, :] = outs[c]
    return full
